# revision 12
# baseline (speedup 1.0000x reference)
"""Multi-head attention (B=4, S=2048, D=1024, H=16) on 8 TRN2 NeuronCores.

Sharding: core c handles batch b = c//2 and head-group hg = c%2 (8 heads).
Tensor-parallel within the core pair of a batch: w_q/w_k/w_v column-split,
w_o row-split; host sums the two partial out-projections per batch.

Device algorithm (per core), all feature-major ("transposed") layouts:
  QhT/KhT = W_slice @ x^T          [512, S]  (pair-slabs of 2 heads x 64)
  Vh      = x @ Wv_aug^T + bv_aug  [S, 520]  (per head: 64 V cols + a const-1
                                              column produced by zero weights
                                              and bias 1.0)
  scoresT[k,q] per head-pair via row-packed matmuls (K=64 each, rows 0-63 /
  64-127 of the PE array), exp on ACT with the 1/8 scale fused, then
  attn@V with stationary [V | 1] (M=65): PSUM row 64 accumulates the softmax
  denominator.  DVE reciprocal + DMA partition-broadcast + DVE multiply
  normalize, and the PE out-projection produces the fp32 partial y^T.
"""

import numpy as np
import ml_dtypes
from contextlib import ExitStack

import concourse.bass as bass
import concourse.tile as tile
from concourse import bacc, mybir
from concourse.bass_utils import run_bass_kernel_spmd

BF16 = ml_dtypes.bfloat16
F32 = np.float32

D = 1024
N_HEAD = 16
DH = 64
HPC = 8          # heads per core
HW = HPC * DH    # head-group width = 512
HA = DH + 1      # per-head augmented width (V + ones col)
P = 128

TRACE = False    # set by test.py for profiling runs
STAGES = 3       # debug: 1=projections, 2=+attention, 3=full
ATT_SUB = 3      # debug: 0=none 1=scores+exp, 2=+attn@V, 3=+division

_PROG = {}


def _build_program(S):
    dt = mybir.dt
    bf = dt.bfloat16
    f32 = dt.float32

    CH = min(512, S)         # q-chunk width (moving free dim)
    NCH = S // CH            # q-chunks
    NT = S // P              # k-tiles (and t-tiles)
    NI = D // P              # contraction tiles over model dim
    NP = HPC // 2            # head pairs
    NOO = D // P             # out-proj o-tiles
    VH = HPC * HA // 2       # 260: half of the augmented V width

    nc = bacc.Bacc("TRN2", target_bir_lowering=False, debug=False)

    xq = nc.dram_tensor("xq", [D, S], bf, kind="ExternalInput")
    xk = nc.dram_tensor("xk", [D, S], bf, kind="ExternalInput")
    xv = nc.dram_tensor("xv", [D, S], bf, kind="ExternalInput")
    wq = nc.dram_tensor("wq", [D, HW], bf, kind="ExternalInput")
    wk = nc.dram_tensor("wk", [D, HW], bf, kind="ExternalInput")
    wv = nc.dram_tensor("wv", [D, HPC * HA], bf, kind="ExternalInput")
    bq = nc.dram_tensor("bq", [P, HW // P], f32, kind="ExternalInput")
    bk = nc.dram_tensor("bk", [P, HW // P], f32, kind="ExternalInput")
    bv = nc.dram_tensor("bv", [1, HPC * HA], bf, kind="ExternalInput")
    wo = nc.dram_tensor("wo", [HW, D], bf, kind="ExternalInput")
    yT = nc.dram_tensor("yT", [D, S], f32, kind="ExternalOutput")

    AF = mybir.ActivationFunctionType

    with tile.TileContext(nc) as tc:
        with ExitStack() as ctx:
            consts = ctx.enter_context(tc.tile_pool(name="consts", bufs=1))
            wpool = ctx.enter_context(tc.tile_pool(name="wpool", bufs=1))
            xpool = ctx.enter_context(tc.tile_pool(name="xpool", bufs=11))
            slabs = ctx.enter_context(tc.tile_pool(name="slabs", bufs=1))
            epool = ctx.enter_context(tc.tile_pool(name="epool", bufs=4))
            dpool = ctx.enter_context(tc.tile_pool(name="dpool", bufs=4))
            spool = ctx.enter_context(tc.tile_pool(name="spool", bufs=3))
            pspair = ctx.enter_context(
                tc.tile_pool(name="pspair", bufs=2, space="PSUM"))
            psacc = ctx.enter_context(
                tc.tile_pool(name="psacc", bufs=4, space="PSUM"))

            # ---- constants ----
            ones_col = consts.tile([1, P], bf)
            nc.vector.memset(ones_col[:], 1.0)
            bq_sb = consts.tile([P, HW // P], f32)
            nc.sync.dma_start(bq_sb[:], bq.ap())
            bk_sb = consts.tile([P, HW // P], f32)
            nc.sync.dma_start(bk_sb[:], bk.ap())
            bv_sb = consts.tile([1, HPC * HA], bf)
            nc.sync.dma_start(bv_sb[:], bv.ap())

            # ---- weights ----
            wq_sb = wpool.tile([P, NI, HW], bf)
            wk_sb = wpool.tile([P, NI, HW], bf)
            wv_sb = wpool.tile([P, NI, HPC * HA], bf)
            wo_sb = wpool.tile([P, HW // P, D], bf)
            for i in range(NI):
                nc.sync.dma_start(wq_sb[:, i, :], wq.ap()[i * P:(i + 1) * P, :])
                nc.sync.dma_start(wk_sb[:, i, :], wk.ap()[i * P:(i + 1) * P, :])
                nc.sync.dma_start(wv_sb[:, i, :], wv.ap()[i * P:(i + 1) * P, :])
            for c in range(HW // P):
                nc.sync.dma_start(wo_sb[:, c, :], wo.ap()[c * P:(c + 1) * P, :])

            # ---- persistent activation slabs ----
            q_slab = slabs.tile([P, NP, S], bf)
            k_slab = slabs.tile([P, NP, S], bf)
            v_sb = slabs.tile([P, NT, HPC * HA], bf)
            attn_sb = slabs.tile([P, NP, S], bf)

            # ---- K and Q projections:  slab[o, t] = sum_i wT[i, o] * xT[i, t]
            for x_dram, w_sb, b_sb, slab in (
                (xk, wk_sb, bk_sb, k_slab),
                (xq, wq_sb, bq_sb, q_slab),
            ):
                xt = [xpool.tile([P, S], bf, tag="x", name=f"xt{i}")
                      for i in range(NI)]
                for i in range(NI):
                    nc.sync.dma_start(xt[i][:], x_dram.ap()[i * P:(i + 1) * P, :])
                for ch in range(NCH):
                    csl = slice(ch * CH, (ch + 1) * CH)
                    for op in range(HW // P // 2):
                        ps = pspair.tile([P, 2 * CH], f32, tag="pair")
                        for half in range(2):
                            o = 2 * op + half
                            for i in range(NI):
                                nc.tensor.matmul(
                                    ps[:, half * CH:(half + 1) * CH],
                                    lhsT=w_sb[:, i, o * P:(o + 1) * P],
                                    rhs=xt[i][:, csl],
                                    start=(i == 0), stop=(i == NI - 1))
                        for half in range(2):
                            o = 2 * op + half
                            nc.scalar.activation(
                                slab[:, o, csl],
                                ps[:, half * CH:(half + 1) * CH],
                                AF.Identity, bias=b_sb[:, o:o + 1])

            # ---- V projection:  v_sb[t, caug] = sum_i xT[i, t] * wv[i, caug] + bv
            xt = [xpool.tile([P, S], bf, tag="x", name=f"xvt{i}")
                  for i in range(NI)]
            for i in range(NI):
                nc.sync.dma_start(xt[i][:], xv.ap()[i * P:(i + 1) * P, :])
            for t in range(NT):
                tsl = slice(t * P, (t + 1) * P)
                ps = pspair.tile([P, 1024], f32, tag="pair")
                for half in range(2):
                    vsl = slice(half * VH, (half + 1) * VH)
                    out = ps[:, half * 512:half * 512 + VH]
                    for i in range(NI):
                        nc.tensor.matmul(
                            out, lhsT=xt[i][:, tsl], rhs=wv_sb[:, i, vsl],
                            start=(i == 0), stop=False)
                    # bias row (also produces the constant-1 columns)
                    nc.tensor.matmul(
                        out, lhsT=ones_col[:], rhs=bv_sb[:, vsl],
                        start=False, stop=True)
                for half in range(2):
                    vsl = slice(half * VH, (half + 1) * VH)
                    nc.vector.tensor_copy(
                        v_sb[:, t, vsl], ps[:, half * 512:half * 512 + VH])

            # ---- attention per (pair, q-chunk) ----
            for p in range(NP if STAGES >= 2 else 0):
                hA, hB = 2 * p, 2 * p + 1
                for ch in range(NCH):
                    csl = slice(ch * CH, (ch + 1) * CH)
                    accA = psacc.tile([P, CH], f32, tag="acc")
                    accB = psacc.tile([P, CH], f32, tag="acc")
                    pend = []  # (exp tile, kt) waiting for their attn@V
                    for kt in range(NT):
                        if ATT_SUB == 0:
                            break
                        ksl = slice(kt * P, (kt + 1) * P)
                        ps = pspair.tile([P, 1024], f32, tag="pair")
                        nc.tensor.matmul(
                            ps[:, 0:CH],
                            lhsT=k_slab[0:64, p, ksl], rhs=q_slab[0:64, p, csl],
                            start=True, stop=True, tile_position=(0, 0))
                        nc.tensor.matmul(
                            ps[:, 512:512 + CH],
                            lhsT=k_slab[64:128, p, ksl], rhs=q_slab[64:128, p, csl],
                            start=True, stop=True, tile_position=(64, 0))
                        et = epool.tile([P, 1024], bf, tag="exp")
                        if CH == 512:
                            nc.scalar.activation(et[:], ps[:], AF.Exp, scale=0.125)
                        else:
                            nc.scalar.activation(et[:, 0:CH], ps[:, 0:CH],
                                                 AF.Exp, scale=0.125)
                            nc.scalar.activation(et[:, 512:512 + CH],
                                                 ps[:, 512:512 + CH],
                                                 AF.Exp, scale=0.125)
                        if ATT_SUB < 2:
                            continue
                        pend.append((et, kt))
                        if len(pend) == 2:
                            _issue_av(nc, pend.pop(0), v_sb, accA, accB,
                                      hA, hB, NT)
                    while pend:
                        _issue_av(nc, pend.pop(0), v_sb, accA, accB, hA, hB, NT)
                    if ATT_SUB < 3:
                        continue

                    # normalize: row 64 of each acc holds the denominator
                    lbt = dpool.tile([P, 2 * CH], f32, tag="lbt")
                    nc.vector.reciprocal(lbt[64:65, 0:CH], accA[64:65, :])
                    nc.vector.reciprocal(lbt[64:65, CH:2 * CH], accB[64:65, :])
                    _bcast_dma(nc, lbt[0:64, 0:CH], lbt[64:65, 0:CH])
                    _bcast_dma(nc, lbt[0:64, CH:2 * CH],
                               lbt[64:65, CH:2 * CH])
                    nc.vector.tensor_mul(
                        attn_sb[0:64, p, csl], accA[0:64, :], lbt[0:64, 0:CH])
                    tmpb = dpool.tile([P, CH], bf, tag="tmpb")
                    nc.vector.tensor_mul(
                        tmpb[0:64, :], accB[0:64, :], lbt[0:64, CH:2 * CH])
                    nc.sync.dma_start(attn_sb[64:128, p, csl], tmpb[0:64, :])

            # ---- out-projection:  yT[o, t] = sum_c woT[c, o] * attn[c, t]
            if STAGES < 3:
                st = spool.tile([P, S], f32, tag="dump")
                nc.vector.tensor_copy(st[:], q_slab[:, 0, :])
                for o in range(NOO):
                    nc.sync.dma_start(yT.ap()[o * P:(o + 1) * P, :], st[:])
            for op in range(NOO // 2 if STAGES >= 3 else 0):
                for ch in range(NCH):
                    csl = slice(ch * CH, (ch + 1) * CH)
                    ps = pspair.tile([P, 2 * CH], f32, tag="pair")
                    for half in range(2):
                        o = 2 * op + half
                        for c in range(HW // P):
                            nc.tensor.matmul(
                                ps[:, half * CH:(half + 1) * CH],
                                lhsT=wo_sb[:, c, o * P:(o + 1) * P],
                                rhs=attn_sb[:, c, csl],
                                start=(c == 0), stop=(c == HW // P - 1))
                    st = spool.tile([P, 2 * CH], f32, tag="stage")
                    nc.vector.tensor_copy(st[:], ps[:])
                    for half in range(2):
                        o = 2 * op + half
                        nc.sync.dma_start(
                            yT.ap()[o * P:(o + 1) * P, csl],
                            st[:, half * CH:(half + 1) * CH])

    nc.compile()
    return nc


def enable_trace():
    """Register the NTFF profiling hook (axon images lack antenv.axon_hooks)
    and neuter the cloud artifact upload; then TRACE=True runs return
    exec_time_ns."""
    global TRACE
    import sys
    import types
    import antenv
    if "antenv.axon_hooks" not in sys.modules:
        _m = types.ModuleType("antenv.axon_hooks")
        _m._hook = None
        _m.set_axon_ntff_profile_hook = lambda h: setattr(_m, "_hook", h)
        _m.get_axon_ntff_profile_hook = lambda: _m._hook
        sys.modules["antenv.axon_hooks"] = _m
        antenv.axon_hooks = _m
        from trn_agent_boot.trn_boot import _ntff_profile_via_ctypes
        _m._hook = _ntff_profile_via_ctypes("/opt/axon/libaxon_pjrt.so")
    import concourse.bass_utils as bu
    bu.upload_artifacts = lambda tmpdir: tmpdir
    TRACE = True


def _bcast_dma(nc, dst, src_row):
    """Broadcast a [1, W] SBUF row to [N, W] via a 0-stride free dim DMA."""
    n = dst.shape[0]
    src_b = bass.AP(tensor=src_row.tensor, offset=src_row.offset,
                    ap=[list(src_row.ap[0]), [0, n], list(src_row.ap[1])])
    nc.sync.dma_start(dst, src_b)


def _issue_av(nc, item, v_sb, accA, accB, hA, hB, NT):
    et, kt = item
    CH = accA.shape[1]
    for acc, h, half in ((accA, hA, 0), (accB, hB, 1)):
        nc.tensor.matmul(
            acc[0:HA, :],
            lhsT=v_sb[:, kt, h * HA:(h + 1) * HA],
            rhs=et[:, half * 512:half * 512 + CH],
            start=(kt == 0), stop=(kt == NT - 1))


def _get_program(S):
    if S not in _PROG:
        _PROG[S] = _build_program(S)
    return _PROG[S]


def _prep_core_inputs(q, k, v, w_q, b_q, w_k, b_k, w_v, b_v, b, hg, S):
    hsl = slice(hg * HW, (hg + 1) * HW)
    wv_aug = np.zeros((D, HPC * HA), F32)
    bv_aug = np.zeros((1, HPC * HA), F32)
    wv_s = w_v[hsl]
    bv_s = b_v[hsl]
    for h in range(HPC):
        wv_aug[:, h * HA:h * HA + DH] = wv_s[h * DH:(h + 1) * DH].T
        bv_aug[0, h * HA:h * HA + DH] = bv_s[h * DH:(h + 1) * DH]
        bv_aug[0, h * HA + DH] = 1.0
    return {
        "xq": np.ascontiguousarray(q[b].T).astype(BF16),
        "xk": np.ascontiguousarray(k[b].T).astype(BF16),
        "xv": np.ascontiguousarray(v[b].T).astype(BF16),
        "wq": np.ascontiguousarray(w_q[hsl].T).astype(BF16),
        "wk": np.ascontiguousarray(w_k[hsl].T).astype(BF16),
        "wv": wv_aug.astype(BF16),
        "bq": np.ascontiguousarray(b_q[hsl].reshape(HW // P, P).T).astype(F32),
        "bk": np.ascontiguousarray(b_k[hsl].reshape(HW // P, P).T).astype(F32),
        "bv": bv_aug.astype(BF16),
        "wo": None,  # filled by caller (needs w_o)
    }


def kernel(q, k, v, w_q, b_q, w_k, b_k, w_v, b_v, w_o, b_o):
    q, k, v = (np.asarray(a, F32) for a in (q, k, v))
    w_q, b_q, w_k, b_k = (np.asarray(a, F32) for a in (w_q, b_q, w_k, b_k))
    w_v, b_v, w_o, b_o = (np.asarray(a, F32) for a in (w_v, b_v, w_o, b_o))
    B, S, _ = q.shape

    nc = _get_program(S)

    n_cores = 2 * B
    in_maps = []
    for c in range(n_cores):
        b, hg = c // 2, c % 2
        m = _prep_core_inputs(q, k, v, w_q, b_q, w_k, b_k, w_v, b_v, b, hg, S)
        hsl = slice(hg * HW, (hg + 1) * HW)
        m["wo"] = np.ascontiguousarray(w_o[:, hsl].T).astype(BF16)
        in_maps.append(m)

    res = run_bass_kernel_spmd(nc, in_maps, list(range(n_cores)), trace=TRACE)

    out = np.empty((B, S, D), F32)
    for b in range(B):
        yt = res.results[2 * b]["yT"] + res.results[2 * b + 1]["yT"]
        out[b] = yt.T + b_o
    if TRACE:
        kernel.last_exec_time_ns = res.exec_time_ns
    return out
